# revision 15
# baseline (speedup 1.0000x reference)
"""Fused multi-head attention block (QKV proj -> 16-head attention -> out proj)
for Trainium2, sharded over 8 NeuronCores.

Sharding: batch (4) x head-halves (2) -> each core handles 1 batch element and
8 of the 16 heads. QKV weights are column-sharded per core's heads, the output
projection is row-sharded (Megatron style); the two partial fc outputs per
batch element are summed on the host (cheap fp32 add) so no collectives are
needed.

Per-core device program (all matmuls bf16 inputs, fp32 PSUM accumulation):
  1a. qkT[f, s] = (w_qk^T x^T)[f, s] + b_qk[f]    f = 8 q-heads*64 ++ 8 k-heads*64
      (q side pre-scaled by 1/sqrt(hd) on host)
  1b. v[s, f] = (x w_v)[s, f] + b_v[f], written into per-head [V | ones]
      augmented lhsT blocks for the AV matmul
  2.  per head h: scoresT[k, q] = kT_h^T-style matmul (contraction hd=64);
      exp on ScalarE straight out of PSUM (no max subtraction -- scores are
      O(1) by construction); AV matmul with the ones column producing the
      softmax denominators in PSUM row 64; normalization via
      reciprocal_approx_fast + DMA partition-broadcast + one DVE multiply.
  3.  fc: attnT (per-head 64-row lhsT) @ w_fc rows for this core's features,
      streamed to DRAM as an fp32 partial.
"""

import sys

if "/opt/trn_rl_repo" not in sys.path:
    sys.path.insert(0, "/opt/trn_rl_repo")

import numpy as np
import ml_dtypes

_BF16 = ml_dtypes.bfloat16

D = 1024
N_HEAD_CORE = 8  # heads per core
HD = 64
F = N_HEAD_CORE * HD  # 512 features per core
VAUG = HD + 1  # V columns + ones column


def _split_excess_waits(nc, limit=1):
    """This container's walrus rejects instructions carrying more than ~2
    semaphore waits ("Too many sync wait commands"). Hoist excess waits onto
    injected same-engine NoOps placed immediately before the instruction —
    sequential waits are semantically identical to one multi-wait."""
    import bass_rust
    import concourse.mybir as mybir

    n_added = 0
    for fn in nc.m.functions:
        for bb in fn.blocks:
            out = []
            changed = False
            for inst in bb.instructions:
                si = inst.sync_info
                waits = list(si.on_wait) if si and si.on_wait else []
                n_upd = len(si.on_update) if si and si.on_update else 0
                allowed = limit
                if len(waits) > allowed and inst.engine != mybir.EngineType.Unassigned:
                    changed = True
                    extra = waits[: len(waits) - allowed]
                    keep = waits[len(waits) - allowed :]
                    for i in range(0, len(extra), limit):
                        nop = mybir.InstNoOp(
                            name=f"waitsplit_{n_added}", ins=[], outs=[]
                        )
                        n_added += 1
                        nop.engine = inst.engine
                        nop.bass_nofuse = True
                        nop.sync_info = bass_rust.SyncInfo(
                            on_wait=extra[i : i + limit], on_update=[]
                        )
                        out.append(nop)
                    inst.sync_info = bass_rust.SyncInfo(
                        on_wait=keep, on_update=list(si.on_update or [])
                    )
                out.append(inst)
            if changed:
                bb.instructions = out
    return n_added


def build_program(S=2048, split_waits=True):
    """Emit the per-core Bass program. S = sequence length (parameterized so a
    shrunk version can run under CoreSim)."""
    import concourse.bass as bass
    import concourse.mybir as mybir
    import concourse.tile as tile

    bf = mybir.dt.bfloat16
    f32 = mybir.dt.float32
    Exp = mybir.ActivationFunctionType.Exp

    QC = min(512, S)  # matmul moving-operand chunk along s/q
    n_qc = S // QC
    n_st = S // 128  # 128-row tiles along s
    n_kc = S // 128  # 128-deep k chunks in attention

    nc = bass.Bass("TRN2", target_bir_lowering=False, debug=False, num_devices=1)

    xT_d = nc.dram_tensor("xT", [D, S], bf, kind="ExternalInput").ap()
    wqk_d = nc.dram_tensor("wqk", [D, 2 * F], bf, kind="ExternalInput").ap()
    bqk_d = nc.dram_tensor("bqk", [128, 8], f32, kind="ExternalInput").ap()
    wv_d = nc.dram_tensor("wv", [D, F], bf, kind="ExternalInput").ap()
    bvb_d = nc.dram_tensor("bvb", [128, F], f32, kind="ExternalInput").ap()
    wfc_d = nc.dram_tensor("wfc", [F, D], bf, kind="ExternalInput").ap()
    out_d = nc.dram_tensor("out", [S, D], f32, kind="ExternalOutput").ap()

    out_t = out_d.rearrange("(st p) n -> st p n", p=128)
    wfc_hview = wfc_d.rearrange("(h p) n -> p h n", p=HD)

    with tile.TileContext(nc) as tc:
        with (
            tc.tile_pool(name="persist", bufs=1) as pp,
            tc.tile_pool(name="evict", bufs=3) as ep,
            tc.tile_pool(name="exp", bufs=2) as xp,
            tc.tile_pool(name="norm", bufs=4) as np_,
            tc.tile_pool(name="dscr", bufs=4, space="DRAM") as dp,
        ):
            xT = pp.tile([128, 8, S], bf)
            wqk = pp.tile([128, 8, 2 * F], bf)
            wv = pp.tile([128, 8, F], bf)
            wfc = pp.tile([HD, N_HEAD_CORE, D], bf)
            bqk = pp.tile([128, 8], f32)
            bvb = pp.tile([128, F], f32)
            qkT = pp.tile([128, 8, S], bf)  # mt 0-3: qT rows, 4-7: kT rows
            vaug = pp.tile([128, n_st, N_HEAD_CORE * VAUG], bf)
            attnT = pp.tile([HD, N_HEAD_CORE, S], bf)

            for kt in range(8):
                nc.sync.dma_start(xT[:, kt, :], xT_d[kt * 128 : (kt + 1) * 128, :])
                nc.sync.dma_start(wqk[:, kt, :], wqk_d[kt * 128 : (kt + 1) * 128, :])
                nc.sync.dma_start(wv[:, kt, :], wv_d[kt * 128 : (kt + 1) * 128, :])
            nc.sync.dma_start(wfc[:], wfc_hview)
            nc.sync.dma_start(bqk[:], bqk_d[:])
            nc.sync.dma_start(bvb[:], bvb_d[:])

            # ---- stage 1a: qkT = w_qk^T @ x^T + b (per-partition bias) ----
            with tc.tile_pool(name="ps1", bufs=4, space="PSUM") as ps1:
                for mt in range(8):
                    for qc in range(n_qc):
                        acc = ps1.tile([128, QC], f32, tag="ps1")
                        for kt in range(8):
                            nc.tensor.matmul(
                                acc[:],
                                wqk[:, kt, mt * 128 : (mt + 1) * 128],
                                xT[:, kt, qc * QC : (qc + 1) * QC],
                                start=(kt == 0),
                                stop=(kt == 7),
                            )
                        nc.vector.tensor_scalar_add(
                            qkT[:, mt, qc * QC : (qc + 1) * QC],
                            acc[:],
                            bqk[:, mt : mt + 1],
                        )

                # ---- stage 1b: v = x @ w_v + b_v -> [V | ones] blocks ----
                # vaug free layout per s-tile: 8 heads x 65 = [V(64) | ones]
                for st in range(n_st):
                    acc = ps1.tile([128, F], f32, tag="ps1")
                    for kt in range(8):
                        nc.tensor.matmul(
                            acc[:],
                            xT[:, kt, st * 128 : (st + 1) * 128],
                            wv[:, kt, :],
                            start=(kt == 0),
                            stop=(kt == 7),
                        )
                    nc.vector.memset(vaug[:, st, :], 1.0)
                    vv = vaug[:, st, :].rearrange("p (h c) -> p h c", c=VAUG)
                    av = acc.rearrange("p (h c) -> p h c", c=HD)
                    bv = bvb.rearrange("p (h c) -> p h c", c=HD)
                    nc.vector.tensor_add(vv[:, :, 0:HD], av[:], bv[:])

            # ---- stage 2: attention per head ----
            with (
                tc.tile_pool(name="ps2s", bufs=1, space="PSUM") as ps2s,
                tc.tile_pool(name="ps2o", bufs=1, space="PSUM") as ps2o,
            ):
                for h in range(N_HEAD_CORE):
                    ht, hp = h // 2, h % 2
                    qs = slice(hp * HD, (hp + 1) * HD)
                    qh = qkT[qs, ht, :]
                    kh = qkT[qs, 4 + ht, :]
                    vcol = h * VAUG

                    ov = ps2o.tile([VAUG, S], f32, tag="O")
                    for kc in range(n_kc):
                        sc = ps2s.tile([128, S], f32, tag="S")
                        for qc in range(n_qc):
                            nc.tensor.matmul(
                                sc[:, qc * QC : (qc + 1) * QC],
                                kh[:, kc * 128 : (kc + 1) * 128],
                                qh[:, qc * QC : (qc + 1) * QC],
                                start=True,
                                stop=True,
                            )
                        e = xp.tile([128, S], bf, tag="exp")
                        nc.scalar.activation(e[:], sc[:], Exp)
                        for qc in range(n_qc):
                            nc.tensor.matmul(
                                ov[:, qc * QC : (qc + 1) * QC],
                                vaug[:, kc, vcol : vcol + VAUG],
                                e[:, qc * QC : (qc + 1) * QC],
                                start=(kc == 0),
                                stop=(kc == n_kc - 1),
                            )
                    # normalize: attnT_h = ov[0:64] / ov[64]. The reciprocal
                    # stays on partition 64 (engines cannot shift partitions);
                    # the partition-broadcast goes through a DRAM scratch row
                    # (step-0 DMA sources must be DRAM).
                    for qc in range(n_qc):
                        rc = np_.tile([HD + 1, QC], f32, tag="recip")
                        nc.vector.reciprocal(
                            rc[HD : HD + 1, :], ov[HD : HD + 1, qc * QC : (qc + 1) * QC]
                        )
                        dr = dp.tile([1, QC], f32, tag="dscr")
                        nc.sync.dma_start(dr[:], rc[HD : HD + 1, :])
                        bc = np_.tile([HD, QC], f32, tag="bcast")
                        nc.sync.dma_start(bc[:], dr[0:1, :].to_broadcast((HD, QC)))
                        nc.vector.tensor_mul(
                            attnT[:, h, qc * QC : (qc + 1) * QC],
                            ov[0:HD, qc * QC : (qc + 1) * QC],
                            bc[:],
                        )

            # ---- stage 3: fc partial = attnT^T @ wfc ----
            with tc.tile_pool(name="ps3", bufs=4, space="PSUM") as ps3:
                for st in range(n_st):
                    for oc in range(2):
                        acc = ps3.tile([128, 512], f32, tag="ps3")
                        for h in range(N_HEAD_CORE):
                            nc.tensor.matmul(
                                acc[:],
                                attnT[:, h, st * 128 : (st + 1) * 128],
                                wfc[:, h, oc * 512 : (oc + 1) * 512],
                                start=(h == 0),
                                stop=(h == N_HEAD_CORE - 1),
                            )
                        o = ep.tile([128, 512], f32, tag="fcout")
                        nc.vector.tensor_copy(o[:], acc[:])
                        nc.sync.dma_start(
                            out_t[st][:, oc * 512 : (oc + 1) * 512], o[:]
                        )

    if split_waits:
        _split_excess_waits(nc)
    return nc


def make_core_inputs(x, w_qkv, b_qkv, w_fc):
    """Shard + lay out host-side inputs for the 8 cores."""
    ins = []
    for core in range(8):
        b, half = core // 2, core % 2
        fsl = slice(half * F, (half + 1) * F)
        w_q = w_qkv[:, 0:D][:, fsl] * np.float32(0.125)
        w_k = w_qkv[:, D : 2 * D][:, fsl]
        w_v = w_qkv[:, 2 * D :][:, fsl]
        b_q = b_qkv[0:D][fsl] * np.float32(0.125)
        b_k = b_qkv[D : 2 * D][fsl]
        b_v = b_qkv[2 * D :][fsl]
        bqk = np.concatenate([b_q, b_k]).astype(np.float32).reshape(8, 128).T
        ins.append(
            {
                "xT": np.ascontiguousarray(x[b].T).astype(_BF16),
                "wqk": np.concatenate([w_q, w_k], axis=1).astype(_BF16),
                "bqk": np.ascontiguousarray(bqk),
                "wv": w_v.astype(_BF16),
                "bvb": np.broadcast_to(b_v.astype(np.float32), (128, F)).copy(),
                "wfc": w_fc[fsl, :].astype(_BF16),
            }
        )
    return ins


_CACHE = {}


def kernel(x, w_qkv, b_qkv, w_fc, b_fc):
    from concourse import bass_utils

    x = np.asarray(x)
    w_qkv = np.asarray(w_qkv)
    b_qkv = np.asarray(b_qkv)
    w_fc = np.asarray(w_fc)
    b_fc = np.asarray(b_fc)
    B, S, _ = x.shape

    if "nc" not in _CACHE:
        _CACHE["nc"] = build_program(S=S)
    nc = _CACHE["nc"]

    in_maps = make_core_inputs(x, w_qkv, b_qkv, w_fc)
    res = bass_utils.run_bass_kernel_spmd(nc, in_maps, core_ids=list(range(8)))
    _CACHE["last_result"] = res

    out = np.empty((B, S, D), dtype=np.float32)
    bfc = b_fc.astype(np.float32)
    for b in range(B):
        out[b] = res.results[2 * b]["out"] + res.results[2 * b + 1]["out"] + bfc
    return out
